# revision 7
# baseline (speedup 1.0000x reference)
"""Full-attention kernel (QKV projections + softmax(QK^T/sqrt(d))V) on 8
trn2 NeuronCores.

Problem: x [2,4096,512] f32, W_{q,k,v} [512,512] f32 -> context [2,4096,512]
f32 (the reference applies no causal mask and dropout=0).

Distribution (data parallel, no collectives -- measured faster than
AllGather-sharded projections on this fabric): core c handles batch
b = c // 4 and query block q0 = (c % 4) * 1024; each core redundantly
projects K^T/V for its whole batch in 4 streamed key passes.  The host
rotates each core's copy of x[b] so its query rows come first (attention
is permutation-invariant over keys, so key order is irrelevant).

Precision: float32r matmuls and PE-transposes (1/1.5 cyc/row), fp32 PSUM
accumulation, fp32 softmax stats; exp on ACT with fused row-sum
(accum_out); no max-subtraction (scores are O(5) by construction);
context normalized by 1/rowsum per query tile as soon as its last key
pass completes.
"""
import numpy as np
from contextlib import ExitStack

from concourse import bacc
import concourse.mybir as mybir
import concourse.tile as tile
from concourse.bass_utils import run_bass_kernel_spmd
from concourse.masks import make_identity

F32 = mybir.dt.float32
F32R = mybir.dt.float32r
AF = mybir.ActivationFunctionType
ADD = mybir.AluOpType.add
AX = mybir.AxisListType

B, S, D = 2, 4096, 512
N_CORES = 8
CORES_PER_B = N_CORES // B
QLEN = S // CORES_PER_B             # 1024
NPASS = 4
KLEN = S // NPASS                   # 1024
P = 128
SCALE = 1.0 / float(np.sqrt(D))

N_QT = QLEN // P                    # 8
N_KB = KLEN // 512                  # 2
N_ST = KLEN // P                    # 8
N_DC = D // P                       # 4


def _build(nreps=1):
    nc = bacc.Bacc(None)
    xb_d = nc.declare_dram_parameter("xb", [S, D], F32R, isOutput=False)
    wq_d = nc.declare_dram_parameter("wq", [D, D], F32R, isOutput=False)
    wk_d = nc.declare_dram_parameter("wk", [D, D], F32R, isOutput=False)
    wv_d = nc.declare_dram_parameter("wv", [D, D], F32R, isOutput=False)
    out_d = nc.declare_dram_parameter("out", [QLEN, D], F32, isOutput=True)

    with tile.TileContext(nc) as tc, ExitStack() as ctx:
        const = ctx.enter_context(tc.tile_pool(name="const", bufs=1))
        w_pool = ctx.enter_context(tc.tile_pool(name="w", bufs=1))
        x_pool = ctx.enter_context(tc.tile_pool(name="x", bufs=4))
        xT_pool = ctx.enter_context(tc.tile_pool(name="xT", bufs=2))
        kT_pool = ctx.enter_context(tc.tile_pool(name="kT", bufs=2))
        v_pool = ctx.enter_context(tc.tile_pool(name="v", bufs=2))
        qT_pool = ctx.enter_context(tc.tile_pool(name="qT", bufs=1))
        pr_pool = ctx.enter_context(tc.tile_pool(name="pr", bufs=4))
        prT_pool = ctx.enter_context(tc.tile_pool(name="prT", bufs=4))
        acc_pool = ctx.enter_context(tc.tile_pool(name="acc", bufs=1))
        st_pool = ctx.enter_context(tc.tile_pool(name="st", bufs=1))

        ps_tr = ctx.enter_context(tc.tile_pool(name="ps_tr", bufs=2, space="PSUM"))
        ps_pj = ctx.enter_context(tc.tile_pool(name="ps_pj", bufs=2, space="PSUM"))
        ps_sc = ctx.enter_context(tc.tile_pool(name="ps_sc", bufs=2, space="PSUM"))
        ps_cx = ctx.enter_context(tc.tile_pool(name="ps_cx", bufs=2, space="PSUM"))

        ident_f = const.tile([P, P], F32)
        make_identity(nc, ident_f[:])
        ident = const.tile([P, P], F32R)
        nc.vector.tensor_copy(ident[:], ident_f[:])
        zbias = const.tile([P, 1], F32)
        nc.vector.memset(zbias[:], 0.0)

        # warm the PE/HAM clock gate with dummy transposes while the first
        # x tiles are still in flight on the DMA queues
        warm = ps_tr.tile([P, N_DC, P], F32R, tag="tr", name="warm")
        for _w in range(16):
            nc.tensor.matmul(warm[:, _w % N_DC, :], ident[:], ident[:],
                             is_transpose=True, start=True, stop=True)

        acc = acc_pool.tile([P, N_QT, D], F32)
        rsums = st_pool.tile([P, N_QT, NPASS * N_KB], F32)
        rtot = st_pool.tile([P, N_QT], F32)
        recip = st_pool.tile([P, N_QT], F32)

        w_tiles = {}

        def emit_W():
            # gpsimd(SWDGE)-issued DMAs: keep SP/ACT queues free for x tiles
            for name, wd in (("wq", wq_d), ("wk", wk_d), ("wv", wv_d)):
                wt = w_pool.tile([P, N_DC, D], F32R, tag=name)
                for c in range(N_DC):
                    nc.gpsimd.dma_start(out=wt[:, c, :],
                                        in_=wd[c * P:(c + 1) * P, :])
                w_tiles[name] = wt

        qT = {}
        kT = {}
        v = {}

        def emit_A(p, first=False):
            r0 = p * KLEN
            xT = xT_pool.tile([P, N_DC, KLEN], F32R, tag="xT")
            for st in range(N_ST):
                x_t = x_pool.tile([P, D], F32R, tag="x")
                xeng = nc.sync if st % 2 == 0 else nc.scalar
                xeng.dma_start(
                    out=x_t[:], in_=xb_d[r0 + st * P:r0 + (st + 1) * P, :])
                if first and st == 0:
                    emit_W()
                ptr = ps_tr.tile([P, N_DC, P], F32R, tag="tr")
                for c in range(N_DC):
                    nc.tensor.matmul(
                        ptr[:, c, :], x_t[:, c * P:(c + 1) * P], ident[:],
                        is_transpose=True, start=True, stop=True)
                nc.scalar.copy(xT[:, :, st * P:(st + 1) * P], ptr[:])
            wq_t, wk_t, wv_t = w_tiles["wq"], w_tiles["wk"], w_tiles["wv"]
            kt = kT_pool.tile([P, N_DC, KLEN], F32R, tag="kT")
            for do in range(N_DC):
                for blk in range(KLEN // 512):
                    pp = ps_pj.tile([P, 512], F32, tag="pj")
                    for c in range(N_DC):
                        nc.tensor.matmul(
                            pp[:], wk_t[:, c, do * P:(do + 1) * P],
                            xT[:, c, blk * 512:(blk + 1) * 512],
                            start=(c == 0), stop=(c == N_DC - 1))
                    nc.vector.tensor_copy(kt[:, do, blk * 512:(blk + 1) * 512],
                                          pp[:])
            vt = v_pool.tile([P, N_ST, D], F32R, tag="v")
            for st in range(N_ST):
                pp = ps_pj.tile([P, 512], F32, tag="pj")
                for c in range(N_DC):
                    nc.tensor.matmul(
                        pp[:], xT[:, c, st * P:(st + 1) * P], wv_t[:, c, :],
                        start=(c == 0), stop=(c == N_DC - 1))
                nc.scalar.copy(vt[:, st, :], pp[:])
            if p == 0:
                qt_ = qT_pool.tile([P, N_DC, QLEN], F32R, tag="qT")
                for do in range(N_DC):
                    for blk in range(QLEN // 512):
                        pp = ps_pj.tile([P, 512], F32, tag="pj")
                        for c in range(N_DC):
                            nc.tensor.matmul(
                                pp[:], wq_t[:, c, do * P:(do + 1) * P],
                                xT[:, c, blk * 512:(blk + 1) * 512],
                                start=(c == 0), stop=(c == N_DC - 1))
                        nc.scalar.mul(qt_[:, do, blk * 512:(blk + 1) * 512],
                                      pp[:], SCALE)
                qT["t"] = qt_
            kT[p] = kt
            v[p] = vt

        def finalize(qt):
            nc.vector.tensor_reduce(rtot[:, qt:qt + 1], rsums[:, qt, :],
                                    axis=AX.X, op=ADD)
            nc.vector.reciprocal(recip[:, qt:qt + 1], rtot[:, qt:qt + 1])
            nc.vector.tensor_scalar_mul(acc[:, qt, :], acc[:, qt, :],
                                        recip[:, qt:qt + 1])
            nc.sync.dma_start(out=out_d[qt * P:(qt + 1) * P, :],
                              in_=acc[:, qt, :])

        def emit_B(p):
            kt, vt, qt_ = kT[p], v[p], qT["t"]
            for qt in range(N_QT):
                pcx = ps_cx.tile([P, D], F32, tag="cx")
                n_mm = N_KB * N_DC
                mm = 0
                for kb in range(N_KB):
                    psc = ps_sc.tile([P, 512], F32, tag="sc")
                    for c in range(N_DC):
                        nc.tensor.matmul(
                            psc[:], qt_[:, c, qt * P:(qt + 1) * P],
                            kt[:, c, kb * 512:(kb + 1) * 512],
                            start=(c == 0), stop=(c == N_DC - 1))
                    probs = pr_pool.tile([P, 512], F32R, tag="pr")
                    nc.scalar.activation(
                        probs[:], psc[:], AF.Exp, bias=zbias[:],
                        accum_out=rsums[:, qt, p * N_KB + kb:p * N_KB + kb + 1])
                    ptr = ps_tr.tile([P, N_DC, P], F32R, tag="tr")
                    for j in range(N_DC):
                        nc.tensor.matmul(
                            ptr[:, j, :], probs[:, j * P:(j + 1) * P], ident[:],
                            is_transpose=True, start=True, stop=True)
                    prT = prT_pool.tile([P, N_DC, P], F32R, tag="prT")
                    nc.vector.tensor_copy(prT[:], ptr[:])
                    for j in range(N_DC):
                        nc.tensor.matmul(
                            pcx[:], prT[:, j, :], vt[:, kb * N_DC + j, :],
                            start=(mm == 0), stop=(mm == n_mm - 1))
                        mm += 1
                if p == 0:
                    nc.vector.tensor_copy(acc[:, qt, :], pcx[:])
                else:
                    nc.vector.tensor_tensor(acc[:, qt, :], pcx[:],
                                            acc[:, qt, :], op=ADD)
                if p == NPASS - 1:
                    finalize(qt)

        for _rep in range(nreps):
            emit_A(0, first=(_rep == 0))
            emit_A(1)
            emit_B(0)
            emit_A(2)
            emit_B(1)
            emit_A(3)
            emit_B(2)
            emit_B(3)

    nc.compile()
    return nc


_CACHE = {}


def _get_nc(nreps=1):
    if nreps not in _CACHE:
        _CACHE[nreps] = _build(nreps)
    return _CACHE[nreps]


def _in_maps(x, W_query, W_key, W_value):
    x = np.ascontiguousarray(np.asarray(x, dtype=np.float32))
    wq = np.ascontiguousarray(np.asarray(W_query, dtype=np.float32))
    wk = np.ascontiguousarray(np.asarray(W_key, dtype=np.float32))
    wv = np.ascontiguousarray(np.asarray(W_value, dtype=np.float32))
    maps = []
    for core in range(N_CORES):
        b = core // CORES_PER_B
        q0 = (core % CORES_PER_B) * QLEN
        xb = np.roll(x[b], -q0, axis=0)
        maps.append({"xb": xb, "wq": wq, "wk": wk, "wv": wv})
    return maps


def kernel(x, W_query, W_key, W_value, _trace=False):
    import os
    if not _trace:
        # NTFF tracing is unavailable here; make sure an inherited
        # BASS_TRACE can't route execution down that path.
        os.environ.setdefault("BASS_NEVER_TRACE", "1")
    nc = _get_nc()
    maps = _in_maps(x, W_query, W_key, W_value)
    res = run_bass_kernel_spmd(nc, maps, list(range(N_CORES)), trace=_trace)
    out = np.empty((B, S, D), dtype=np.float32)
    for core in range(N_CORES):
        b = core // CORES_PER_B
        q0 = (core % CORES_PER_B) * QLEN
        out[b, q0:q0 + QLEN] = res.results[core]["out"]
    if _trace:
        return out, res
    return out


# revision 9
# speedup vs baseline: 1.0323x; 1.0323x over previous
"""Full-attention kernel (QKV projections + softmax(QK^T/sqrt(d))V) on 8
trn2 NeuronCores.

Problem: x [2,4096,512] f32, W_{q,k,v} [512,512] f32 -> context [2,4096,512]
f32 (the reference applies no causal mask and dropout=0).

Distribution (data parallel, no collectives -- measured faster than
AllGather-sharded projections on this fabric): core c handles batch
b = c // 4 and query block q0 = (c % 4) * 1024; each core redundantly
projects K^T/V for its whole batch in 4 streamed key passes.  The host
rotates each core's copy of x[b] so its query rows come first (attention
is permutation-invariant over keys, so key order is irrelevant).

Precision: float32r matmuls and PE-transposes (1/1.5 cyc/row), fp32 PSUM
accumulation, fp32 softmax stats; exp on ACT with fused row-sum
(accum_out); no max-subtraction (scores are O(5) by construction);
context normalized by 1/rowsum per query tile as soon as its last key
pass completes.
"""
import numpy as np
from contextlib import ExitStack

from concourse import bacc
import concourse.mybir as mybir
import concourse.tile as tile
from concourse.bass_utils import run_bass_kernel_spmd
from concourse.masks import make_identity

F32 = mybir.dt.float32
F32R = mybir.dt.float32r
BF16 = mybir.dt.bfloat16
AF = mybir.ActivationFunctionType
ADD = mybir.AluOpType.add
AX = mybir.AxisListType

B, S, D = 2, 4096, 512
N_CORES = 8
CORES_PER_B = N_CORES // B
QLEN = S // CORES_PER_B             # 1024
NPASS = 4
KLEN = S // NPASS                   # 1024
P = 128
SCALE = 1.0 / float(np.sqrt(D))

N_QT = QLEN // P                    # 8
N_KB = KLEN // 512                  # 2
N_ST = KLEN // P                    # 8
N_DC = D // P                       # 4


def _build(nreps=1):
    nc = bacc.Bacc(None)
    xb_d = nc.declare_dram_parameter("xb", [S, D], F32R, isOutput=False)
    wq_d = nc.declare_dram_parameter("wq", [D, D], F32R, isOutput=False)
    wk_d = nc.declare_dram_parameter("wk", [D, D], F32R, isOutput=False)
    wv_d = nc.declare_dram_parameter("wv", [D, D], F32R, isOutput=False)
    out_d = nc.declare_dram_parameter("out", [QLEN, D], F32, isOutput=True)

    with tile.TileContext(nc) as tc, ExitStack() as ctx:
        const = ctx.enter_context(tc.tile_pool(name="const", bufs=1))
        w_pool = ctx.enter_context(tc.tile_pool(name="w", bufs=1))
        x_pool = ctx.enter_context(tc.tile_pool(name="x", bufs=4))
        xT_pool = ctx.enter_context(tc.tile_pool(name="xT", bufs=2))
        kT_pool = ctx.enter_context(tc.tile_pool(name="kT", bufs=2))
        v_pool = ctx.enter_context(tc.tile_pool(name="v", bufs=2))
        qT_pool = ctx.enter_context(tc.tile_pool(name="qT", bufs=1))
        pr_pool = ctx.enter_context(tc.tile_pool(name="pr", bufs=4))
        prT_pool = ctx.enter_context(tc.tile_pool(name="prT", bufs=4))
        acc_pool = ctx.enter_context(tc.tile_pool(name="acc", bufs=1))
        st_pool = ctx.enter_context(tc.tile_pool(name="st", bufs=1))

        ps_tr = ctx.enter_context(tc.tile_pool(name="ps_tr", bufs=2, space="PSUM"))
        ps_pj = ctx.enter_context(tc.tile_pool(name="ps_pj", bufs=2, space="PSUM"))
        ps_sc = ctx.enter_context(tc.tile_pool(name="ps_sc", bufs=2, space="PSUM"))
        ps_cx = ctx.enter_context(tc.tile_pool(name="ps_cx", bufs=2, space="PSUM"))

        ident_f = const.tile([P, P], F32)
        make_identity(nc, ident_f[:])
        ident = const.tile([P, P], F32R)
        nc.vector.tensor_copy(ident[:], ident_f[:])
        ident_bf = const.tile([P, P], BF16)
        nc.vector.tensor_copy(ident_bf[:], ident_f[:])
        zbias = const.tile([P, 1], F32)
        nc.vector.memset(zbias[:], 0.0)

        # warm the PE/HAM clock gate with dummy transposes while the first
        # x tiles are still in flight on the DMA queues
        warm = ps_tr.tile([P, N_DC, P], F32R, tag="tr", name="warm")
        for _w in range(16):
            nc.tensor.matmul(warm[:, _w % N_DC, :], ident[:], ident[:],
                             is_transpose=True, start=True, stop=True)

        acc = acc_pool.tile([P, N_QT, D], F32)
        rsums = st_pool.tile([P, N_QT, NPASS * N_KB], F32)
        rtot = st_pool.tile([P, N_QT], F32)
        recip = st_pool.tile([P, N_QT], F32)

        w_tiles = {}

        def emit_W():
            # gpsimd(SWDGE)-issued DMAs: keep SP/ACT queues free for x tiles
            for name, wd in (("wq", wq_d), ("wk", wk_d), ("wv", wv_d)):
                wt = w_pool.tile([P, N_DC, D], F32R, tag=name)
                for c in range(N_DC):
                    nc.gpsimd.dma_start(out=wt[:, c, :],
                                        in_=wd[c * P:(c + 1) * P, :])
                w_tiles[name] = wt

        qT = {}
        kT = {}
        v = {}

        def emit_A(p, first=False):
            r0 = p * KLEN
            xT = xT_pool.tile([P, N_DC, KLEN], F32R, tag="xT")
            for st in range(N_ST):
                x_t = x_pool.tile([P, D], F32R, tag="x")
                xeng = nc.sync if st % 2 == 0 else nc.scalar
                if first and st < 2:
                    # column-chunk loads so the first transposes start sooner
                    for c in range(N_DC):
                        xeng.dma_start(
                            out=x_t[:, c * P:(c + 1) * P],
                            in_=xb_d[r0 + st * P:r0 + (st + 1) * P,
                                     c * P:(c + 1) * P])
                else:
                    xeng.dma_start(
                        out=x_t[:], in_=xb_d[r0 + st * P:r0 + (st + 1) * P, :])
                if first and st == 0:
                    emit_W()
                ptr = ps_tr.tile([P, N_DC, P], F32R, tag="tr")
                for c in range(N_DC):
                    nc.tensor.matmul(
                        ptr[:, c, :], x_t[:, c * P:(c + 1) * P], ident[:],
                        is_transpose=True, start=True, stop=True)
                nc.scalar.copy(xT[:, :, st * P:(st + 1) * P], ptr[:])
            wq_t, wk_t, wv_t = w_tiles["wq"], w_tiles["wk"], w_tiles["wv"]
            kt = kT_pool.tile([P, N_DC, KLEN], F32R, tag="kT")
            for do in range(N_DC):
                for blk in range(KLEN // 512):
                    pp = ps_pj.tile([P, 512], F32, tag="pj")
                    for c in range(N_DC):
                        nc.tensor.matmul(
                            pp[:], wk_t[:, c, do * P:(do + 1) * P],
                            xT[:, c, blk * 512:(blk + 1) * 512],
                            start=(c == 0), stop=(c == N_DC - 1))
                    nc.vector.tensor_copy(kt[:, do, blk * 512:(blk + 1) * 512],
                                          pp[:])
            vt = v_pool.tile([P, N_ST, D], BF16, tag="v")
            for st in range(N_ST):
                pp = ps_pj.tile([P, 512], F32, tag="pj")
                for c in range(N_DC):
                    nc.tensor.matmul(
                        pp[:], xT[:, c, st * P:(st + 1) * P], wv_t[:, c, :],
                        start=(c == 0), stop=(c == N_DC - 1))
                nc.scalar.copy(vt[:, st, :], pp[:])
            if p == 0:
                qt_ = qT_pool.tile([P, N_DC, QLEN], F32R, tag="qT")
                for do in range(N_DC):
                    for blk in range(QLEN // 512):
                        pp = ps_pj.tile([P, 512], F32, tag="pj")
                        for c in range(N_DC):
                            nc.tensor.matmul(
                                pp[:], wq_t[:, c, do * P:(do + 1) * P],
                                xT[:, c, blk * 512:(blk + 1) * 512],
                                start=(c == 0), stop=(c == N_DC - 1))
                        nc.scalar.mul(qt_[:, do, blk * 512:(blk + 1) * 512],
                                      pp[:], SCALE)
                qT["t"] = qt_
            kT[p] = kt
            v[p] = vt

        def finalize(qt):
            nc.vector.tensor_reduce(rtot[:, qt:qt + 1], rsums[:, qt, :],
                                    axis=AX.X, op=ADD)
            nc.vector.reciprocal(recip[:, qt:qt + 1], rtot[:, qt:qt + 1])
            nc.vector.tensor_scalar_mul(acc[:, qt, :], acc[:, qt, :],
                                        recip[:, qt:qt + 1])
            nc.sync.dma_start(out=out_d[qt * P:(qt + 1) * P, :],
                              in_=acc[:, qt, :])

        def emit_B(p):
            kt, vt, qt_ = kT[p], v[p], qT["t"]
            for qt in range(N_QT):
                pcx = ps_cx.tile([P, D], F32, tag="cx")
                n_mm = N_KB * N_DC
                mm = 0
                for kb in range(N_KB):
                    psc = ps_sc.tile([P, 512], F32, tag="sc")
                    for c in range(N_DC):
                        nc.tensor.matmul(
                            psc[:], qt_[:, c, qt * P:(qt + 1) * P],
                            kt[:, c, kb * 512:(kb + 1) * 512],
                            start=(c == 0), stop=(c == N_DC - 1))
                    probs = pr_pool.tile([P, 512], BF16, tag="pr")
                    nc.scalar.activation(
                        probs[:], psc[:], AF.Exp, bias=zbias[:],
                        accum_out=rsums[:, qt, p * N_KB + kb:p * N_KB + kb + 1])
                    ptr = ps_tr.tile([P, N_DC, P], BF16, tag="tr")
                    for j in range(N_DC):
                        nc.tensor.matmul(
                            ptr[:, j, :], probs[:, j * P:(j + 1) * P], ident_bf[:],
                            is_transpose=True, start=True, stop=True)
                    prT = prT_pool.tile([P, N_DC, P], BF16, tag="prT")
                    nc.vector.tensor_copy(prT[:], ptr[:])
                    for j in range(N_DC):
                        nc.tensor.matmul(
                            pcx[:], prT[:, j, :], vt[:, kb * N_DC + j, :],
                            start=(mm == 0), stop=(mm == n_mm - 1))
                        mm += 1
                if p == 0:
                    nc.vector.tensor_copy(acc[:, qt, :], pcx[:])
                else:
                    nc.vector.tensor_tensor(acc[:, qt, :], pcx[:],
                                            acc[:, qt, :], op=ADD)
                if p == NPASS - 1:
                    finalize(qt)

        for _rep in range(nreps):
            emit_A(0, first=(_rep == 0))
            emit_A(1)
            emit_B(0)
            emit_A(2)
            emit_B(1)
            emit_A(3)
            emit_B(2)
            emit_B(3)

    nc.compile()
    return nc


_CACHE = {}


def _get_nc(nreps=1):
    if nreps not in _CACHE:
        _CACHE[nreps] = _build(nreps)
    return _CACHE[nreps]


def _in_maps(x, W_query, W_key, W_value):
    x = np.ascontiguousarray(np.asarray(x, dtype=np.float32))
    wq = np.ascontiguousarray(np.asarray(W_query, dtype=np.float32))
    wk = np.ascontiguousarray(np.asarray(W_key, dtype=np.float32))
    wv = np.ascontiguousarray(np.asarray(W_value, dtype=np.float32))
    maps = []
    for core in range(N_CORES):
        b = core // CORES_PER_B
        q0 = (core % CORES_PER_B) * QLEN
        xb = np.roll(x[b], -q0, axis=0)
        maps.append({"xb": xb, "wq": wq, "wk": wk, "wv": wv})
    return maps


def kernel(x, W_query, W_key, W_value, _trace=False):
    import os
    if not _trace:
        # NTFF tracing is unavailable here; make sure an inherited
        # BASS_TRACE can't route execution down that path.
        os.environ.setdefault("BASS_NEVER_TRACE", "1")
    nc = _get_nc()
    maps = _in_maps(x, W_query, W_key, W_value)
    res = run_bass_kernel_spmd(nc, maps, list(range(N_CORES)), trace=_trace)
    out = np.empty((B, S, D), dtype=np.float32)
    for core in range(N_CORES):
        b = core // CORES_PER_B
        q0 = (core % CORES_PER_B) * QLEN
        out[b, q0:q0 + QLEN] = res.results[core]["out"]
    if _trace:
        return out, res
    return out
